# revision 18
# baseline (speedup 1.0000x reference)
"""Trainium2 Bass kernel for the 3-head GCN block (v12, ~98.5us).

Structure per core (16 samples, processed as 8 pairs of 2):
  conv stage   PE: per 5t-chunk, z[tu, (h,a,o)] = x-chunk^T @ wdt (N=384),
               lhsT padded to 128 cols (junk rows nullified by bd's zero
               pad rows) so every zt tile is fully written and LDWEIGHTS
               gets fast-weight-load.
  copy stage   ACT/DVE alternate strictly: zt PSUM->SBUF f32->bf16
               [128,768] per chunk-pair. This is the hard wall: PSUM is
               read at 1 f32/cycle/partition and only ACT+DVE have PSUM
               ports (PSUM->SBUF DMA is not supported; GpSimd has no PSUM
               port and its elementwise ops measure ~100x slower).
  A stage      PE: per chunk-pair 6 matmuls (2 chunks x 3 heads) accumulate
               g[(a,o),(t,v)] += zt^T @ block-diag(A_h) (N=125), flushed a
               FIFO depth of 3 chunk-pairs behind the conv so the copy's
               sem+queue+copy latency (~1.2-1.9us) is hidden; the FIFO is
               carried across pair boundaries so the pipeline never drains.
  residual     PE: one identity-diag matmul per macro (g starts at x; BN
               scale is folded into Wd on the host so no scale op needed).
  epilogue     relu(g+sh) in ONE pass: ACT activation(Relu, bias=sh) and
               DVE tensor_scalar(add sh, max 0), alternating per macro.
  HAM warmup   40 dummy matmuls on a memset tile fill the ~10us runtime
               preamble + DMA rampup so the PE clock gate is at 2.4GHz
               (not 1.2) when real work arrives.
  head/tail    pair-0 macro-0 split [2,2,3,3,5,5] so the first conv needs
               only 180 cols of x; last pair writes output per-macro.

x and out move over HBM as bf16 (host-side cast, 13.1MB/core total).
Engine budget per pair: PE ~10.2us, ACT ~8.2us, DVE ~8.7us; wall is
PE-span + ~6us fixed preamble + ~4us fixed teardown. rel err ~4.1e-3.
"""

import numpy as np
import ml_dtypes

import concourse.bass as bass
import concourse.tile as tile
from concourse import bacc, mybir
from concourse import bass_utils

BN_EPS = 1e-5

N, C, T, V, H = 128, 64, 128, 25, 3
NCORES = 8
NS = N // NCORES
NPAIRS = NS // 2

TSZ = 5
CHUNKS = [(i * TSZ, TSZ) for i in range(24)] + [(120, 4), (124, 4)]
MACROS = [CHUNKS[i : i + 4] for i in range(0, 24, 4)] + [CHUNKS[24:]]
# pair 0 only: first macro split fine so the opening conv needs just 180
# cols of x (chunk pairs must have equal width)
MACRO0_P0 = [(0, 2), (2, 2), (4, 3), (7, 3), (10, 5), (15, 5)]
FLUSH_DEPTH = 3

_CACHE = {}


def _build_nc():
    f32 = mybir.dt.float32
    bf16 = mybir.dt.bfloat16
    add = mybir.AluOpType.add
    amax = mybir.AluOpType.max
    relu = mybir.ActivationFunctionType.Relu

    nc = bacc.Bacc("TRN2", target_bir_lowering=False, debug=False)

    x_d = nc.dram_tensor("x", (NS, C, T * V), bf16, kind="ExternalInput").ap()
    cc_d = nc.dram_tensor("cc", (128, 512), bf16, kind="ExternalInput").ap()
    bd_d = nc.dram_tensor("bd", (128, 3, TSZ * V), bf16, kind="ExternalInput").ap()
    sh_d = nc.dram_tensor("sh", (128, 1), f32, kind="ExternalInput").ap()
    out_d = nc.dram_tensor("out", (NS, C, T * V), bf16, kind="ExternalOutput").ap()

    with tile.TileContext(nc) as tc:
        with (
            tc.tile_pool(name="consts", bufs=1) as consts,
            tc.tile_pool(name="xo", bufs=3) as xo,
            tc.tile_pool(name="rp", bufs=2) as rp,
            tc.tile_pool(name="zt", bufs=6) as ztp,
            tc.tile_pool(name="ps_zt", bufs=3, space="PSUM") as ps_zt,
            tc.tile_pool(name="ps_g", bufs=2, space="PSUM") as ps_g,
        ):
            # HAM warmup: the PE sits idle ~10us during the runtime preamble
            # and DMA-queue rampup, then runs its first ~3.4us of real work
            # at 1.2GHz (clock gate). Fill the dead head with dummy matmuls
            # on a memset tile (no DMA dependency) so the gate flips to
            # 2.4GHz before real work arrives.
            ms_sb = consts.tile([128, 128], bf16)
            nc.vector.memset(ms_sb[:], 0.25)
            warm_ps = ps_zt.tile([128, 2, 4, 2, 64], f32, tag="zt_ps")
            for _ in range(40):
                nc.tensor.matmul(
                    warm_ps[:, 0, 0, :, :],
                    lhsT=ms_sb[:],
                    rhs=ms_sb[:],
                    start=True,
                    stop=True,
                )
            # first x slice goes out before anything else on SP: the first
            # (128-col padded) conv needs only cols 0:180
            xb_cur = xo.tile([128, T * V], bf16, tag="xb")
            x0_dram = x_d[0:2].rearrange("a c f -> (a c) f")
            W0 = sum(tsz for _, tsz in MACRO0_P0) * V
            nc.sync.dma_start(out=xb_cur[:, 0:180], in_=x0_dram[:, 0:180])
            cc_sb = consts.tile([128, 512], bf16)
            nc.sync.dma_start(out=cc_sb[:], in_=cc_d[:])
            bd_sb = consts.tile([128, 3, TSZ * V], bf16)
            nc.sync.dma_start(out=bd_sb[:], in_=bd_d[:])
            nc.sync.dma_start(out=xb_cur[:, 180:W0], in_=x0_dram[:, 180:W0])
            # sh is only needed by the first epilogue (~10us in); issue it
            # from the Activation HWDGE queue to keep SP free for x
            sh_sb = consts.tile([128, 1], f32)
            nc.scalar.dma_start(out=sh_sb[:], in_=sh_d[:])
            wdt_sb = cc_sb[:, 0:384]
            id_sb = cc_sb[:, 384:512]

            def xdram(p):
                return x_d[2 * p : 2 * p + 2].rearrange("a c f -> (a c) f")

            # rest of pair-0's input, per-macro slices (slice 0 went first)
            for macro in MACROS[1:]:
                t0m = macro[0][0]
                W = sum(tsz for _, tsz in macro) * V
                sl = slice(t0m * V, t0m * V + W)
                nc.sync.dma_start(out=xb_cur[:, sl], in_=xdram(0)[:, sl])

            def epilogue(eng, r_tile, pg_ps, psl):
                if eng is nc.scalar:
                    nc.scalar.activation(
                        r_tile[:, psl], pg_ps[:], relu, bias=sh_sb[:]
                    )
                else:
                    nc.vector.tensor_scalar(
                        r_tile[:, psl], pg_ps[:], sh_sb[:], 0.0,
                        op0=add, op1=amax,
                    )

            flush_q = []

            def flush_one():
                job = flush_q.pop(0)
                pg, offs, pMc, pzt, epi = job
                for k in range(2):
                    for grp in range(3):
                        nc.tensor.matmul(
                            pg[:, offs[k] : offs[k] + pMc],
                            lhsT=pzt[:, k, grp],
                            rhs=bd_sb[:, grp, :pMc],
                            start=False,
                            stop=(epi is not None and k == 1 and grp == 2),
                        )
                if epi is not None:
                    eng, r_tile, psl, o_dram, lastp = epi
                    epilogue(eng, r_tile, pg, psl)
                    if lastp:
                        nc.sync.dma_start(out=o_dram[:, psl], in_=r_tile[:, psl])
                    elif psl.stop == T * V:
                        # last macro of a non-final pair: whole-pair output
                        nc.sync.dma_start(out=o_dram[:], in_=r_tile[:])

            for p in range(NPAIRS):
                macros = ([MACRO0_P0] + MACROS[1:]) if p == 0 else MACROS
                copy_eng = [nc.vector, nc.scalar]
                epi_eng = [nc.scalar, nc.vector]

                o_dram = out_d[2 * p : 2 * p + 2].rearrange("a c f -> (a c) f")
                xb_tile = xb_cur
                # prefetch next pair's input now - one full pair ahead of use
                if p + 1 < NPAIRS:
                    xb_cur = xo.tile([128, T * V], bf16, tag="xb")
                    nc.sync.dma_start(out=xb_cur[:], in_=xdram(p + 1)[:])
                r_tile = rp.tile([128, T * V], bf16, tag="r")

                ci = 0
                ei = 0
                for mi, macro in enumerate(macros):
                    t0m = macro[0][0]
                    W = sum(tsz for _, tsz in macro) * V
                    g_ps = ps_g.tile([128, W], f32, tag="g_ps")
                    pairs = [macro[i : i + 2] for i in range(0, len(macro), 2)]
                    for cpi, cpair in enumerate(pairs):
                        Mc = cpair[0][1] * V
                        zt_ps = ps_zt.tile([128, 2, 4, 2, 64], f32, tag="zt_ps")
                        for k, (t0, tsz) in enumerate(cpair):
                            # pad the stationary operand to 128 cols (reads
                            # neighboring x; junk rows die against bd's zero
                            # pad rows) -> full zt tile write + FWL
                            Mk = min(128, T * V - t0 * V)
                            nc.tensor.matmul(
                                zt_ps[:Mk, k, 0:3, :, :],
                                lhsT=xb_tile[:, t0 * V : t0 * V + Mk],
                                rhs=wdt_sb,
                                start=True,
                                stop=True,
                            )
                        if cpi == 0:
                            # residual opener after the first convs
                            nc.tensor.matmul(
                                g_ps[:],
                                lhsT=id_sb,
                                rhs=xb_tile[:, t0m * V : t0m * V + W],
                                start=True,
                                stop=False,
                            )
                        zt_sb = ztp.tile([128, 2, 3, 2, 64], bf16, tag="zt_sb")
                        eng = copy_eng[ci % 2]
                        src = zt_ps[:, :, 0:3, :, :]
                        dst = zt_sb[:]
                        if eng is nc.scalar:
                            nc.scalar.copy(dst, src)
                        else:
                            eng.tensor_copy(dst, src)
                        ci += 1
                        offs = [(t0 - t0m) * V for t0, _ in cpair]
                        epi = None
                        if cpair is pairs[-1]:
                            psl = slice(t0m * V, t0m * V + W)
                            epi = (epi_eng[ei % 2], r_tile, psl, o_dram,
                                   p == NPAIRS - 1)
                            ei += 1
                        flush_q.append((g_ps, offs, Mc, zt_sb, epi))
                        if len(flush_q) > FLUSH_DEPTH:
                            flush_one()
            while flush_q:
                flush_one()

    nc.compile()
    return nc


def _get_nc():
    if "nc" not in _CACHE:
        _CACHE["nc"] = _build_nc()
    return _CACHE["nc"]


def _host_consts(A, Wd, bd, gamma, beta, run_mean, run_var):
    A = np.asarray(A, np.float32)
    Wd = np.asarray(Wd, np.float32)
    bd = np.asarray(bd, np.float32)
    gamma = np.asarray(gamma, np.float32)
    beta = np.asarray(beta, np.float32)
    run_mean = np.asarray(run_mean, np.float32)
    run_var = np.asarray(run_var, np.float32)

    scale = gamma / np.sqrt(run_var + BN_EPS)  # (64,)
    shift = (bd.sum(axis=0) - run_mean) * scale + beta  # (64,)

    cc = np.zeros((128, 512), np.float32)
    wdt2 = cc[:, 0:384].reshape(128, 3, 2, 64)
    for h in range(H):
        # BN scale folded into the conv weights: columns scaled by scale[o]
        wds = Wd[h].T * scale[None, :]  # [c, o]
        wdt2[0:64, h, 0, :] = wds
        wdt2[64:128, h, 1, :] = wds
    # identity block for the residual opener
    cc[:, 384:512][np.arange(128), np.arange(128)] = 1.0

    M = TSZ * V
    # bd padded to 128 contraction rows; rows 125:128 stay zero so the
    # padded conv's junk zt rows contribute nothing
    bdm = np.zeros((128, 3, M), np.float32)
    for h in range(H):
        for i in range(TSZ):
            bdm[i * 25 : (i + 1) * 25, h, i * 25 : (i + 1) * 25] = A[h].T
    bdm = bdm.astype(ml_dtypes.bfloat16)
    ccb = cc.astype(ml_dtypes.bfloat16)

    sh2 = np.tile(shift, 2)[:, None].astype(np.float32)
    return ccb, bdm, sh2


def _in_maps(x, A, Wd, bd, gamma, beta, run_mean, run_var):
    x = np.asarray(x, np.float32).reshape(N, C, T * V)
    xb = np.ascontiguousarray(x).astype(ml_dtypes.bfloat16)
    ccb, bdm, sh2 = _host_consts(A, Wd, bd, gamma, beta, run_mean, run_var)
    return [
        {
            "x": xb[i * NS : (i + 1) * NS],
            "cc": ccb,
            "bd": bdm,
            "sh": sh2,
        }
        for i in range(NCORES)
    ]


def kernel(x, A, Wd, bd, gamma, beta, run_mean, run_var, _trace=False):
    nc = _get_nc()
    in_maps = _in_maps(x, A, Wd, bd, gamma, beta, run_mean, run_var)
    res = bass_utils.run_bass_kernel_spmd(
        nc, in_maps, core_ids=list(range(NCORES)), trace=_trace
    )
    out = np.concatenate(
        [
            np.asarray(r["out"]).astype(np.float32).reshape(NS, C, T, V)
            for r in res.results
        ],
        axis=0,
    )
    _CACHE["last_results"] = res
    return out
